# revision 18
# baseline (speedup 1.0000x reference)
"""Trainium2 Bass kernel for the Diffusion get_energy problem.

Math (per graph b, all computed on one NeuronCore; data-parallel over the
8 graphs across 8 cores):

  rot = QR(pre_rot).Q                        (host, tiny)
  new_lig[t,l] = rot[t] @ lig_coord[l] + trans[t]          (host, tiny)
  atn[l,r,e]  = sum_f lig_feat[l,e,f]*rec_feat[r,e,f] * mask[l,r]   (PE)
  d2[t,l,r]   = |new_lig[t,l] - rec_coord[r]|^2            (PE matmul)
  U[b,t] = sum_{l,r,e} atn[l,r,e] * d(t,l,r)^exps[e],  exps=[-3,-2,-1,1,2]

Precision strategy:
  - atn features for the power channels in fp8e4m3 (DoubleRow matmuls);
    features are ~N(0,0.1) and the error averages out over F=512.
  - d2 via a 3-way bf16 split of the K=5 contraction (kept cross terms
    hh/hm/mh/mm/hl/lh -> K=30, all-bf16, fp32 PSUM accumulate) for
    ~fp32-accurate d2 at bf16 matmul speed (1 cyc/col).
  - channel +2 (dominant) analytic with bf16 atn2 and fp32 W/u2 chain.

Engine assignment per timestep:
  PE   : d2 (K=30 bf16) + one-hot reduction of all four channel products
         (lig mask rides the one-hot columns)
  DVE  : s2 = 1/d2 (custom RECIPROCAL_APPROX_FAST from PSUM, bf16 out),
         one tensor_tensor for the ch2-tail/ch1/chd products (bf16 2x)
  Act  : s1 = sqrt(s2), d1 = sqrt(d2)  (single sqrt table set)
  Pool : s3 = s2*s1, p3 = atn3*s3, p2-head = atn2*s2
"""

import numpy as np
import ml_dtypes

B, T, L, R, E, F = 8, 16, 128, 1024, 5, 512
KF = F // 128  # 4 f-blocks of 128
NCHIP = 8
XSPL = 128  # ch2 product columns handled by DVE (tail); rest on Pool

_BUILT = None  # cached (nc, meta)


# --------------------------------------------------------------------------
# device program
# --------------------------------------------------------------------------
def build_nc(repeat=1):
    from contextlib import ExitStack

    import concourse.bacc as bacc
    import concourse.mybir as mybir
    import concourse.tile as tile

    f32 = mybir.dt.float32
    bf16 = mybir.dt.bfloat16
    fp8 = mybir.dt.float8e4
    AF = mybir.ActivationFunctionType
    MUL = mybir.AluOpType.mult
    DR = mybir.MatmulPerfMode.DoubleRow

    nc = bacc.Bacc("TRN2", target_bir_lowering=False)

    d_ligTb = nc.dram_tensor("ligTb", [128, 4 * KF * L], fp8, kind="ExternalInput")
    d_ligT4 = nc.dram_tensor("ligT4", [128, KF * L], bf16, kind="ExternalInput")
    d_recTb = nc.dram_tensor("recTb", [128, 4 * KF * R], fp8, kind="ExternalInput")
    d_recT4 = nc.dram_tensor("recT4", [128, KF * R], bf16, kind="ExternalInput")
    d_nlaug = nc.dram_tensor("nlaug", [30, T * L], bf16, kind="ExternalInput")
    d_recaug = nc.dram_tensor("recaug", [30, R], bf16, kind="ExternalInput")
    d_nl2d = nc.dram_tensor("nl2d", [128, 5 * T], f32, kind="ExternalInput")
    d_ydev = nc.dram_tensor("ydev", [128, 8 * 5], f32, kind="ExternalInput")
    d_onehot = nc.dram_tensor("onehot", [128, T * T], bf16, kind="ExternalInput")
    d_u4 = nc.dram_tensor("u4", [16, 1], f32, kind="ExternalOutput")
    d_u2 = nc.dram_tensor("u2", [1, 16], f32, kind="ExternalOutput")

    with ExitStack() as ctx:
        tc = ctx.enter_context(tile.TileContext(nc))
        const = ctx.enter_context(tc.tile_pool(name="const", bufs=1 if repeat == 1 else 2))
        recp = ctx.enter_context(tc.tile_pool(name="recp", bufs=3))
        dcp = ctx.enter_context(tc.tile_pool(name="dcp", bufs=6))
        pcp = ctx.enter_context(tc.tile_pool(name="pcp", bufs=3))
        psA = ctx.enter_context(tc.tile_pool(name="psA", bufs=1, space="PSUM"))
        psD = ctx.enter_context(tc.tile_pool(name="psD", bufs=2, space="PSUM"))
        psU = ctx.enter_context(tc.tile_pool(name="psU", bufs=2, space="PSUM"))
        psX = ctx.enter_context(tc.tile_pool(name="psX", bufs=1, space="PSUM"))

        for _rep in range(repeat):
            # ---- input DMAs: all heavy loads on the SP queue (it has no
            # compute), ordered by first consumption ---------------------------
            t_nlaug = const.tile([30, T * L], bf16)
            nc.sync.dma_start(out=t_nlaug[:, 0 : 2 * L], in_=d_nlaug[:, 0 : 2 * L])
            t_recaug = const.tile([30, R], bf16)
            nc.sync.dma_start(out=t_recaug[:], in_=d_recaug[:])
            t_onehot = const.tile([128, T * T], bf16)
            nc.sync.dma_start(out=t_onehot[:], in_=d_onehot[:])
            t_ligTb = const.tile([128, 4 * KF, L], fp8)
            nc.sync.dma_start(out=t_ligTb[:], in_=d_ligTb[:])
            nc.sync.dma_start(
                out=t_nlaug[:, 2 * L : T * L], in_=d_nlaug[:, 2 * L : T * L]
            )
            t_ydev = const.tile([128, 8 * 5], f32)
            t_nl2d = const.tile([128, 5 * T], f32)

            # ---- t-loop machinery --------------------------------------------
            t_upsum = psU.tile([16, 512], f32)

            def emit_d2(t, ps):
                # K=30 bf16 contraction: 3-way bf16 split of the 5-dim dot
                # (kept cross terms give ~fp32-accurate d2 at 1 cyc/col)
                for h in range(2):
                    nc.tensor.matmul(
                        ps[:, h * 512 : (h + 1) * 512],
                        lhsT=t_nlaug[:, t * L : (t + 1) * L],
                        rhs=t_recaug[:, h * 512 : (h + 1) * 512],
                        start=True,
                        stop=True,
                    )

            from concourse.dve_ops import (
                RECIP_APPROX_FAST_CONSTS,
                RECIPROCAL_APPROX_FAST,
            )

            rc = RECIP_APPROX_FAST_CONSTS

            def produce_strips(ps):
                """Emit recip + 2 sqrts + s3 for one timestep's d2 PSUM tile."""
                t_dcat = dcp.tile([128, 4 * R], bf16, tag="dcat")
                s3 = t_dcat[:, 0 * R : 1 * R]
                s2 = t_dcat[:, 1 * R : 2 * R]
                s1 = t_dcat[:, 2 * R : 3 * R]
                d1 = t_dcat[:, 3 * R : 4 * R]
                # s2 strip = 1/d2 via the fast custom DVE reciprocal, written
                # bf16 directly (input must be fp32; output cast is fine)
                nc.vector._custom_dve(
                    RECIPROCAL_APPROX_FAST,
                    out=s2,
                    in0=ps[:],
                    s0=rc["s0"],
                    s1=rc["s1"],
                    imm2=rc["imm2"],
                )
                # d = sqrt(d2) straight from PSUM; s = sqrt(1/d2)
                nc.scalar.activation(out=d1, in_=ps[:], func=AF.Sqrt)
                nc.scalar.activation(out=s1, in_=s2, func=AF.Sqrt)
                nc.gpsimd.tensor_tensor(out=s3, in0=s2, in1=s1, op=MUL)
                return t_dcat

            # prologue: d2 + strips for t=0,1 ahead of the atn phase so the
            # DVE/Act/Pool strip pipeline fills while PE chews on atn matmuls
            ps_d2 = psD.tile([128, 1024], f32, tag="d2")
            emit_d2(0, ps_d2)
            dc_cur = produce_strips(ps_d2)
            ps_d2 = psD.tile([128, 1024], f32, tag="d2")
            emit_d2(1, ps_d2)
            dc_next = produce_strips(ps_d2)

            # ---- atn coefficients ---------------------------------------------
            # channels 0..3 -> bf16 cat buffer (strip order matches exps order
            # [-3,-2,-1,+1]); fp8 features, DoubleRow matmuls (2 k-tiles/pass).
            # rec mask is pre-applied to recTb/recT4 on the host, lig mask rides
            # in the one-hot reduction columns, so these are plain copies.
            t_atncat = const.tile([128, 4 * R], bf16)
            for e in range(4):
                t_rec = recp.tile([128, KF, R], fp8, tag="rec")
                nc.sync.dma_start(
                    out=t_rec[:],
                    in_=d_recTb[:, e * KF * R : (e + 1) * KF * R],
                )
                for h in range(2):
                    ps_a = psA.tile([128, 512], f32, tag="atn")
                    for k in range(0, KF, 2):
                        nc.tensor.matmul(
                            ps_a[:],
                            lhsT=t_ligTb[:, e * KF + k : e * KF + k + 2, :],
                            rhs=t_rec[:, k : k + 2, h * 512 : h * 512 + 512],
                            start=(k == 0),
                            stop=(k == KF - 2),
                            perf_mode=DR,
                        )
                    dst = t_atncat[:, e * R + h * 512 : e * R + h * 512 + 512]
                    if h == 0:
                        nc.scalar.copy(out=dst, in_=ps_a[:])
                    else:
                        nc.vector.tensor_copy(dst, ps_a[:])

            t_rec4 = recp.tile([128, KF * R], bf16, tag="rec4")
            nc.sync.dma_start(out=t_rec4[:], in_=d_recT4[:])
            t_ligT4 = const.tile([128, KF * L], bf16)
            nc.sync.dma_start(out=t_ligT4[:], in_=d_ligT4[:])
            nc.sync.dma_start(out=t_ydev[:], in_=d_ydev[:])
            nc.sync.dma_start(out=t_nl2d[:], in_=d_nl2d[:])

            # ---- t-loop: powers + products + reduction -------------------------
            # software-pipelined emission: d2(t+2) is issued mid-t so every
            # engine's stream stays ahead of its consumers.
            for t in range(T):
                if t + 2 < T:
                    ps_d2 = psD.tile([128, 1024], f32, tag="d2")
                    emit_d2(t + 2, ps_d2)
                    dc_next2 = produce_strips(ps_d2)
                else:
                    dc_next2 = None
                # products: Pool takes ch3 + the head of ch2; DVE takes the
                # tail of ch2 + ch1 + chd in one bf16-2x tensor_tensor
                t_p = pcp.tile([128, 4 * R], bf16, tag="pcat")
                nc.gpsimd.tensor_tensor(
                    out=t_p[:, 0:R],
                    in0=t_atncat[:, 0:R],
                    in1=dc_cur[:, 0:R],
                    op=MUL,
                )
                nc.gpsimd.tensor_tensor(
                    out=t_p[:, R : 2 * R - XSPL],
                    in0=t_atncat[:, R : 2 * R - XSPL],
                    in1=dc_cur[:, R : 2 * R - XSPL],
                    op=MUL,
                )
                nc.vector.tensor_tensor(
                    out=t_p[:, 2 * R - XSPL : 4 * R],
                    in0=t_atncat[:, 2 * R - XSPL : 4 * R],
                    in1=dc_cur[:, 2 * R - XSPL : 4 * R],
                    op=MUL,
                )
                for c in range(8):
                    nc.tensor.matmul(
                        t_upsum[:],
                        lhsT=t_onehot[:, t * T : (t + 1) * T],
                        rhs=t_p[:, c * 512 : (c + 1) * 512],
                        start=(t == 0 and c == 0),
                        stop=(t == T - 1 and c == 7),
                    )
                dc_cur = dc_next
                dc_next = dc_next2
            t_u4 = const.tile([16, 1], f32)
            nc.vector.tensor_reduce(
                out=t_u4[:],
                in_=t_upsum[:],
                axis=mybir.AxisListType.X,
                op=mybir.AluOpType.add,
            )
            nc.gpsimd.dma_start(out=d_u4[:], in_=t_u4[:])

            # ---- analytic +2 channel (emitted last: PE is idle-ish during the
            # t-loop, and u2 is independent of the power channels) -------------
            # operand-swapped matmuls give atn2^T [r,l] directly (no transposes);
            # rec mask is folded into ydev, lig mask into nl2d (both host-side).
            t_atn2T = const.tile([128, R], f32)
            for rk in range(8):
                ps_t = psX.tile([128, 128], f32, tag="aux")
                for k in range(KF):
                    nc.tensor.matmul(
                        ps_t[:],
                        lhsT=t_rec4[:, k * R + rk * 128 : k * R + (rk + 1) * 128],
                        rhs=t_ligT4[:, k * L : (k + 1) * L],
                        start=(k == 0),
                        stop=(k == KF - 1),
                    )
                if rk % 2 == 0:
                    nc.scalar.copy(
                        out=t_atn2T[:, rk * 128 : (rk + 1) * 128], in_=ps_t[:]
                    )
                else:
                    nc.vector.tensor_copy(
                        t_atn2T[:, rk * 128 : (rk + 1) * 128], ps_t[:]
                    )
            ps_w = psX.tile([128, 5], f32, tag="aux")
            for rk in range(8):
                nc.tensor.matmul(
                    ps_w[:],
                    lhsT=t_atn2T[:, rk * 128 : (rk + 1) * 128],
                    rhs=t_ydev[:, rk * 5 : (rk + 1) * 5],
                    start=(rk == 0),
                    stop=(rk == 7),
                )
            t_w = const.tile([128, 5], f32)
            nc.scalar.copy(out=t_w[:], in_=ps_w[:])
            ps_u2 = psX.tile([1, 16], f32, tag="aux")
            for c in range(5):
                nc.tensor.matmul(
                    ps_u2[:],
                    lhsT=t_w[:, c : c + 1],
                    rhs=t_nl2d[:, c * T : (c + 1) * T],
                    start=(c == 0),
                    stop=(c == 4),
                )
            t_u2 = const.tile([1, 16], f32)
            nc.scalar.copy(out=t_u2[:], in_=ps_u2[:])
            nc.gpsimd.dma_start(out=d_u2[:], in_=t_u2[:])

    nc.compile()

    # All activation funcs used here (Sqrt, Copy) live together in the
    # sqrt_and_others table set; dedupe the auto-inserted loads down to a
    # single load of that set (all have empty sync_info, so deletion is safe).
    from concourse.hw_specs import get_activation_tables

    set_names = list(get_activation_tables(nc.m.arch).keys())
    target = set_names.index("sqrt_and_others")
    kept = False
    for blk in nc.m.functions[0].blocks:
        out = []
        for inst in blk.instructions:
            if isinstance(inst, mybir.InstLoadActFuncSet):
                si = inst.sync_info
                empty = si is None or (not si.on_wait and not si.on_update)
                if not kept or not empty:
                    inst.act_func_set_id = target
                    out.append(inst)
                    kept = True
            else:
                out.append(inst)
        blk.instructions[:] = out
    return nc


# --------------------------------------------------------------------------
# host-side data prep
# --------------------------------------------------------------------------
def prep_core_inputs(
    b, lig_feat, rec_feat, lig_coord, rec_coord, rot, trans, lig_counts, rec_counts
):
    """Build the in_map for core b (all numpy)."""
    f32 = np.float32
    lc = np.asarray(lig_coord[b], f32)  # [L,3]
    rc = np.asarray(rec_coord[b], f32)  # [R,3]
    new_lig = (
        np.einsum("tij,lj->tli", np.asarray(rot[b], f32), lc)
        + np.asarray(trans[b], f32)[:, None, :]
    )  # [T,L,3]
    nl2 = (new_lig.astype(f32) ** 2).sum(-1).astype(f32)  # [T,L]
    rec2 = (rc**2).sum(-1).astype(f32)  # [R]

    nlaug = np.empty((5, T * L), f32)
    nlaug[0:3] = new_lig.transpose(2, 0, 1).reshape(3, T * L)
    nlaug[3] = nl2.reshape(-1)
    nlaug[4] = 1.0

    recaug = np.empty((5, R), f32)
    recaug[0:3] = -2.0 * rc.T
    recaug[3] = 1.0
    recaug[4] = rec2

    # 3-way bf16 split of the K=5 contraction: keep the hh/hm/mh/mm/hl/lh
    # cross terms (K=30, all-bf16 operands, fp32 PSUM accumulate) for
    # ~fp32-accurate d2 at bf16 matmul speed.
    def _split3(x):
        bfd = ml_dtypes.bfloat16
        h = x.astype(bfd).astype(f32)
        m = (x - h).astype(bfd).astype(f32)
        lo = (x - h - m).astype(bfd)
        return h.astype(bfd), m.astype(bfd), lo

    ha, ma, la = _split3(nlaug)
    hb, mb, lb = _split3(recaug)
    nlaug30 = np.concatenate([ha, ha, ma, ma, ha, la], 0)  # [30, T*L]
    recaug30 = np.concatenate([hb, mb, hb, mb, lb, hb], 0)  # [30, R]

    ligm = (np.arange(L) < int(lig_counts[b])).astype(f32)
    recm = (np.arange(R) < int(rec_counts[b])).astype(f32)

    fp8 = ml_dtypes.float8_e4m3fn
    lt = np.asarray(lig_feat[b], f32).transpose(1, 2, 0)  # [E,F,L]
    ligT = lt.reshape(E, KF, 128, L).transpose(2, 0, 1, 3)  # [128,E,KF,L]
    ligTb = np.ascontiguousarray(ligT[:, 0:4]).reshape(128, 4 * KF * L)
    ligTb = ligTb.astype(fp8)
    ligT4 = np.ascontiguousarray(ligT[:, 4]).reshape(128, KF * L)
    ligT4 = ligT4.astype(ml_dtypes.bfloat16)
    rt = np.asarray(rec_feat[b], f32).transpose(1, 2, 0)  # [E,F,R]
    recT = rt.reshape(E, KF, 128, R).transpose(2, 0, 1, 3)  # [128,E,KF,R]
    # rec mask pre-applied to the bf16 channels (so atn needs no device mask)
    recTb = np.ascontiguousarray(recT[:, 0:4] * recm).reshape(128, 4 * KF * R)
    recTb = recTb.astype(fp8)
    recT4 = np.ascontiguousarray(recT[:, 4] * recm).reshape(128, KF * R)
    recT4 = recT4.astype(ml_dtypes.bfloat16)

    # lig mask folded into nl2d columns (zeroes padded-l terms of U2)
    nl2d = np.empty((128, 5, T), f32)
    nl2d[:, 0:3, :] = (-2.0 * new_lig).transpose(1, 2, 0)
    nl2d[:, 3, :] = nl2.T
    nl2d[:, 4, :] = 1.0
    nl2d *= ligm[:, None, None]
    nl2d = nl2d.reshape(128, 5 * T)

    # rec mask folded into ydev rows (zeroes padded-r terms of W)
    y = np.empty((R, 5), f32)
    y[:, 0:3] = rc
    y[:, 3] = 1.0
    y[:, 4] = rec2
    y *= recm[:, None]
    ydev = np.ascontiguousarray(y.reshape(8, 128, 5).transpose(1, 0, 2)).reshape(
        128, 40
    )

    # lig mask folded into the one-hot reduction columns
    oh = np.zeros((128, T, T), f32)
    oh[:, np.arange(T), np.arange(T)] = ligm[:, None]
    onehot = oh.reshape(128, T * T).astype(ml_dtypes.bfloat16)

    return {
        "ligTb": ligTb,
        "ligT4": ligT4,
        "recTb": recTb,
        "recT4": recT4,
        "nlaug": nlaug30,
        "recaug": recaug30,
        "nl2d": nl2d,
        "ydev": ydev,
        "onehot": onehot,
    }


def host_rot(pre_rot):
    return np.linalg.qr(np.asarray(pre_rot, np.float32))[0]


# --------------------------------------------------------------------------
# entry point
# --------------------------------------------------------------------------
def kernel(
    lig_feat, rec_feat, lig_coord, rec_coord, pre_rot, trans, lig_counts, rec_counts
):
    global _BUILT
    from concourse.bass_utils import run_bass_kernel_spmd

    if _BUILT is None:
        _BUILT = build_nc()
    nc = _BUILT

    rot = host_rot(pre_rot)
    in_maps = [
        prep_core_inputs(
            b,
            lig_feat,
            rec_feat,
            lig_coord,
            rec_coord,
            rot,
            trans,
            lig_counts,
            rec_counts,
        )
        for b in range(B)
    ]
    res = run_bass_kernel_spmd(nc, in_maps, core_ids=list(range(NCHIP))).results
    out = np.empty((B, T), np.float32)
    for b in range(B):
        out[b] = res[b]["u4"][:, 0] + res[b]["u2"][0, :]
    return out


# revision 25
# speedup vs baseline: 1.6163x; 1.6163x over previous
"""Trainium2 Bass kernel for the Diffusion get_energy problem.

Math (per graph b, all computed on one NeuronCore; data-parallel over the
8 graphs across 8 cores):

  rot = QR(pre_rot).Q                        (host, tiny)
  new_lig[t,l] = rot[t] @ lig_coord[l] + trans[t]          (host, tiny)
  atn[l,r,e]  = sum_f lig_feat[l,e,f]*rec_feat[r,e,f] * mask[l,r]   (PE)
  d2[t,l,r]   = |new_lig[t,l] - rec_coord[r]|^2            (PE matmul)
  U[b,t] = sum_{l,r,e} atn[l,r,e] * d(t,l,r)^exps[e],  exps=[-3,-2,-1,1,2]

Precision strategy:
  - atn features for the power channels in fp8e4m3 (DoubleRow matmuls);
    features are ~N(0,0.1) and the error averages out over F=512.
  - d2 via a 3-way bf16 split of the K=5 contraction (kept cross terms
    hh/hm/mh/mm/hl/lh -> K=30, all-bf16, fp32 PSUM accumulate) for
    ~fp32-accurate d2 at bf16 matmul speed (1 cyc/col).
  - channel +2 (dominant) analytic with bf16 atn2 and fp32 W/u2 chain.

Engine assignment per timestep:
  PE   : d2 (K=30 bf16) + one-hot reduction of all four channel products
         (lig mask rides the one-hot columns)
  DVE  : s2 = 1/d2 (custom RECIPROCAL_APPROX_FAST from PSUM, bf16 out),
         one tensor_tensor for the ch2-tail/ch1/chd products (bf16 2x)
  Act  : s1 = sqrt(s2), d1 = sqrt(d2)  (single sqrt table set)
  Pool : s3 = s2*s1, p3 = atn3*s3, p2-head = atn2*s2
"""

import numpy as np
import ml_dtypes

B, T, L, R, E, F = 8, 16, 128, 1024, 5, 512
KF = F // 128  # 4 f-blocks of 128
NCHIP = 8
XSPL = 128  # ch2 product columns handled by DVE (tail); rest on Pool

_BUILT = None  # cached (nc, meta)


# --------------------------------------------------------------------------
# device program
# --------------------------------------------------------------------------
def build_nc(repeat=1):
    from contextlib import ExitStack

    import concourse.bacc as bacc
    import concourse.mybir as mybir
    import concourse.tile as tile

    f32 = mybir.dt.float32
    bf16 = mybir.dt.bfloat16
    fp8 = mybir.dt.float8e4
    AF = mybir.ActivationFunctionType
    MUL = mybir.AluOpType.mult
    DR = mybir.MatmulPerfMode.DoubleRow

    nc = bacc.Bacc("TRN2", target_bir_lowering=False)

    d_ligTb = nc.dram_tensor("ligTb", [128, 4 * KF * L], fp8, kind="ExternalInput")
    d_ligT4 = nc.dram_tensor("ligT4", [128, KF * L], bf16, kind="ExternalInput")
    d_recTb = nc.dram_tensor("recTb", [128, 4 * KF * R], fp8, kind="ExternalInput")
    d_recT4 = nc.dram_tensor("recT4", [128, KF * R], bf16, kind="ExternalInput")
    d_nlaug = nc.dram_tensor("nlaug", [30, T * L], bf16, kind="ExternalInput")
    d_recaug = nc.dram_tensor("recaug", [30, R], bf16, kind="ExternalInput")
    d_nl2d = nc.dram_tensor("nl2d", [128, 5 * T], f32, kind="ExternalInput")
    d_ydev = nc.dram_tensor("ydev", [128, 8 * 5], f32, kind="ExternalInput")
    d_onehot = nc.dram_tensor("onehot", [128, T * T], bf16, kind="ExternalInput")
    d_u4 = nc.dram_tensor("u4", [16, 1], f32, kind="ExternalOutput")
    d_u2 = nc.dram_tensor("u2", [1, 16], f32, kind="ExternalOutput")

    with ExitStack() as ctx:
        tc = ctx.enter_context(tile.TileContext(nc))
        const = ctx.enter_context(tc.tile_pool(name="const", bufs=1 if repeat == 1 else 2))
        recp = ctx.enter_context(tc.tile_pool(name="recp", bufs=4))
        dcp = ctx.enter_context(tc.tile_pool(name="dcp", bufs=8))
        pcp = ctx.enter_context(tc.tile_pool(name="pcp", bufs=3))
        psA = ctx.enter_context(tc.tile_pool(name="psA", bufs=1, space="PSUM"))
        psD = ctx.enter_context(tc.tile_pool(name="psD", bufs=2, space="PSUM"))
        psU = ctx.enter_context(tc.tile_pool(name="psU", bufs=2, space="PSUM"))

        for _rep in range(repeat):
            # ---- input DMAs: all heavy loads on the SP queue (it has no
            # compute), ordered by first consumption ---------------------------
            t_nlaug = const.tile([30, T * L], bf16)
            nc.sync.dma_start(out=t_nlaug[:, 0 : 2 * L], in_=d_nlaug[:, 0 : 2 * L])
            t_recaug = const.tile([30, R], bf16)
            nc.sync.dma_start(out=t_recaug[:], in_=d_recaug[:])
            t_onehot = const.tile([128, T * T], bf16)
            nc.sync.dma_start(out=t_onehot[:], in_=d_onehot[:])
            t_ligTb = const.tile([128, 4 * KF, L], fp8)
            nc.sync.dma_start(out=t_ligTb[:], in_=d_ligTb[:])
            nc.sync.dma_start(
                out=t_nlaug[:, 2 * L : T * L], in_=d_nlaug[:, 2 * L : T * L]
            )
            t_ydev = const.tile([128, 8 * 5], f32)
            t_nl2d = const.tile([128, 5 * T], f32)

            # ---- t-loop machinery --------------------------------------------
            t_upsum = psU.tile([16, 512], f32)

            def emit_d2(t, ps):
                # K=30 bf16 contraction: 3-way bf16 split of the 5-dim dot
                # (kept cross terms give ~fp32-accurate d2 at 1 cyc/col)
                for h in range(2):
                    nc.tensor.matmul(
                        ps[:, h * 512 : (h + 1) * 512],
                        lhsT=t_nlaug[:, t * L : (t + 1) * L],
                        rhs=t_recaug[:, h * 512 : (h + 1) * 512],
                        start=True,
                        stop=True,
                    )

            from concourse.dve_ops import (
                RECIP_APPROX_FAST_CONSTS,
                RECIPROCAL_APPROX_FAST,
            )

            rc = RECIP_APPROX_FAST_CONSTS

            def produce_strips(ps):
                """Emit recip + 2 sqrts + s3 for one timestep's d2 PSUM tile."""
                t_dcat = dcp.tile([128, 4 * R], bf16, tag="dcat")
                s3 = t_dcat[:, 0 * R : 1 * R]
                s2 = t_dcat[:, 1 * R : 2 * R]
                s1 = t_dcat[:, 2 * R : 3 * R]
                d1 = t_dcat[:, 3 * R : 4 * R]
                # s2 strip = 1/d2 via the fast custom DVE reciprocal, written
                # bf16 directly (input must be fp32; output cast is fine)
                nc.vector._custom_dve(
                    RECIPROCAL_APPROX_FAST,
                    out=s2,
                    in0=ps[:],
                    s0=rc["s0"],
                    s1=rc["s1"],
                    imm2=rc["imm2"],
                )
                # d = sqrt(d2) straight from PSUM; s = sqrt(1/d2)
                nc.scalar.activation(out=d1, in_=ps[:], func=AF.Sqrt)
                nc.scalar.activation(out=s1, in_=s2, func=AF.Sqrt)
                nc.gpsimd.tensor_tensor(out=s3, in0=s2, in1=s1, op=MUL)
                return t_dcat

            # prologue: d2 + strips for t=0,1 ahead of the atn phase so the
            # DVE/Act/Pool strip pipeline fills while PE chews on atn matmuls
            ps_d2 = psD.tile([128, 1024], f32, tag="d2")
            emit_d2(0, ps_d2)
            dc_cur = produce_strips(ps_d2)
            ps_d2 = psD.tile([128, 1024], f32, tag="d2")
            emit_d2(1, ps_d2)
            dc_next = produce_strips(ps_d2)

            # ---- atn coefficients ---------------------------------------------
            # channels 0..3 -> bf16 cat buffer (strip order matches exps order
            # [-3,-2,-1,+1]); fp8 features, DoubleRow matmuls (2 k-tiles/pass).
            # rec mask is pre-applied to recTb/recT4 on the host, lig mask rides
            # in the one-hot reduction columns, so these are plain copies.
            t_atncat = const.tile([128, 4 * R], bf16)
            for e in range(4):
                t_rec = recp.tile([128, KF, R], fp8, tag="rec")
                nc.sync.dma_start(
                    out=t_rec[:],
                    in_=d_recTb[:, e * KF * R : (e + 1) * KF * R],
                )
                for h in range(2):
                    ps_a = psA.tile([128, 512], f32, tag="atn")
                    for k in range(0, KF, 2):
                        nc.tensor.matmul(
                            ps_a[:],
                            lhsT=t_ligTb[:, e * KF + k : e * KF + k + 2, :],
                            rhs=t_rec[:, k : k + 2, h * 512 : h * 512 + 512],
                            start=(k == 0),
                            stop=(k == KF - 2),
                            perf_mode=DR,
                        )
                    dst = t_atncat[:, e * R + h * 512 : e * R + h * 512 + 512]
                    if (e * 2 + h) % 8 < 5:
                        nc.scalar.copy(out=dst, in_=ps_a[:])
                    else:
                        nc.vector.tensor_copy(dst, ps_a[:])

            t_rec4 = recp.tile([128, KF * R], bf16, tag="rec4")
            nc.sync.dma_start(out=t_rec4[:], in_=d_recT4[:])
            t_ligT4 = const.tile([128, KF * L], bf16)
            nc.sync.dma_start(out=t_ligT4[:], in_=d_ligT4[:])
            nc.sync.dma_start(out=t_ydev[:], in_=d_ydev[:])
            nc.sync.dma_start(out=t_nl2d[:], in_=d_nl2d[:])

            # ---- t-loop: powers + products + reduction -------------------------
            # software-pipelined emission: d2(t+2) is issued mid-t so every
            # engine's stream stays ahead of its consumers.
            for t in range(T):
                if t + 2 < T:
                    ps_d2 = psD.tile([128, 1024], f32, tag="d2")
                    emit_d2(t + 2, ps_d2)
                    dc_next2 = produce_strips(ps_d2)
                else:
                    dc_next2 = None
                # products: Pool takes ch3 + the head of ch2; DVE takes the
                # tail of ch2 + ch1 + chd in one bf16-2x tensor_tensor
                t_p = pcp.tile([128, 4 * R], bf16, tag="pcat")
                nc.gpsimd.tensor_tensor(
                    out=t_p[:, 0:R],
                    in0=t_atncat[:, 0:R],
                    in1=dc_cur[:, 0:R],
                    op=MUL,
                )
                nc.gpsimd.tensor_tensor(
                    out=t_p[:, R : 2 * R - XSPL],
                    in0=t_atncat[:, R : 2 * R - XSPL],
                    in1=dc_cur[:, R : 2 * R - XSPL],
                    op=MUL,
                )
                nc.vector.tensor_tensor(
                    out=t_p[:, 2 * R - XSPL : 4 * R],
                    in0=t_atncat[:, 2 * R - XSPL : 4 * R],
                    in1=dc_cur[:, 2 * R - XSPL : 4 * R],
                    op=MUL,
                )
                for c in range(8):
                    nc.tensor.matmul(
                        t_upsum[:],
                        lhsT=t_onehot[:, t * T : (t + 1) * T],
                        rhs=t_p[:, c * 512 : (c + 1) * 512],
                        start=(t == 0 and c == 0),
                        stop=(t == T - 1 and c == 7),
                    )
                dc_cur = dc_next
                dc_next = dc_next2
            t_u4 = const.tile([16, 1], f32)
            t_udummy = pcp.tile([16, 512], bf16, tag="ud")
            nc.scalar.activation(
                out=t_udummy[:], in_=t_upsum[:], func=AF.Copy,
                accum_out=t_u4[:],
            )
            nc.gpsimd.dma_start(out=d_u4[:], in_=t_u4[:])

            # ---- analytic +2 channel (emitted last: PE is idle-ish during the
            # t-loop, and u2 is independent of the power channels) -------------
            # operand-swapped matmuls give atn2^T [r,l] directly (no transposes);
            # rec mask is folded into ydev, lig mask into nl2d (both host-side).
            t_atn2T = const.tile([128, R], f32)
            for rk in range(8):
                ps_t = psA.tile([128, 128], f32, tag="aux")
                for k in range(KF):
                    nc.tensor.matmul(
                        ps_t[:],
                        lhsT=t_rec4[:, k * R + rk * 128 : k * R + (rk + 1) * 128],
                        rhs=t_ligT4[:, k * L : (k + 1) * L],
                        start=(k == 0),
                        stop=(k == KF - 1),
                    )
                if rk % 2 == 0:
                    nc.scalar.copy(
                        out=t_atn2T[:, rk * 128 : (rk + 1) * 128], in_=ps_t[:]
                    )
                else:
                    nc.vector.tensor_copy(
                        t_atn2T[:, rk * 128 : (rk + 1) * 128], ps_t[:]
                    )
            ps_w = psA.tile([128, 5], f32, tag="aux")
            for rk in range(8):
                nc.tensor.matmul(
                    ps_w[:],
                    lhsT=t_atn2T[:, rk * 128 : (rk + 1) * 128],
                    rhs=t_ydev[:, rk * 5 : (rk + 1) * 5],
                    start=(rk == 0),
                    stop=(rk == 7),
                )
            t_w = const.tile([128, 5], f32)
            nc.scalar.copy(out=t_w[:], in_=ps_w[:])
            ps_u2 = psA.tile([1, 16], f32, tag="aux")
            for c in range(5):
                nc.tensor.matmul(
                    ps_u2[:],
                    lhsT=t_w[:, c : c + 1],
                    rhs=t_nl2d[:, c * T : (c + 1) * T],
                    start=(c == 0),
                    stop=(c == 4),
                )
            t_u2 = const.tile([1, 16], f32)
            nc.scalar.copy(out=t_u2[:], in_=ps_u2[:])
            nc.gpsimd.dma_start(out=d_u2[:], in_=t_u2[:])

    nc.compile()

    # All activation funcs used here (Sqrt, Copy) live together in the
    # sqrt_and_others table set; dedupe the auto-inserted loads down to a
    # single load of that set (all have empty sync_info, so deletion is safe).
    from concourse.hw_specs import get_activation_tables

    set_names = list(get_activation_tables(nc.m.arch).keys())
    target = set_names.index("sqrt_and_others")
    kept = False
    for blk in nc.m.functions[0].blocks:
        out = []
        for inst in blk.instructions:
            if isinstance(inst, mybir.InstLoadActFuncSet):
                si = inst.sync_info
                empty = si is None or (not si.on_wait and not si.on_update)
                if not kept or not empty:
                    inst.act_func_set_id = target
                    out.append(inst)
                    kept = True
            else:
                out.append(inst)
        blk.instructions[:] = out
    return nc


# --------------------------------------------------------------------------
# host-side data prep
# --------------------------------------------------------------------------
def prep_core_inputs(
    b, lig_feat, rec_feat, lig_coord, rec_coord, rot, trans, lig_counts, rec_counts
):
    """Build the in_map for core b (all numpy)."""
    f32 = np.float32
    lc = np.asarray(lig_coord[b], f32)  # [L,3]
    rc = np.asarray(rec_coord[b], f32)  # [R,3]
    new_lig = (
        np.einsum("tij,lj->tli", np.asarray(rot[b], f32), lc)
        + np.asarray(trans[b], f32)[:, None, :]
    )  # [T,L,3]
    nl2 = (new_lig.astype(f32) ** 2).sum(-1).astype(f32)  # [T,L]
    rec2 = (rc**2).sum(-1).astype(f32)  # [R]

    nlaug = np.empty((5, T * L), f32)
    nlaug[0:3] = new_lig.transpose(2, 0, 1).reshape(3, T * L)
    nlaug[3] = nl2.reshape(-1)
    nlaug[4] = 1.0

    recaug = np.empty((5, R), f32)
    recaug[0:3] = -2.0 * rc.T
    recaug[3] = 1.0
    recaug[4] = rec2

    # 3-way bf16 split of the K=5 contraction: keep the hh/hm/mh/mm/hl/lh
    # cross terms (K=30, all-bf16 operands, fp32 PSUM accumulate) for
    # ~fp32-accurate d2 at bf16 matmul speed.
    def _split3(x):
        bfd = ml_dtypes.bfloat16
        h = x.astype(bfd).astype(f32)
        m = (x - h).astype(bfd).astype(f32)
        lo = (x - h - m).astype(bfd)
        return h.astype(bfd), m.astype(bfd), lo

    ha, ma, la = _split3(nlaug)
    hb, mb, lb = _split3(recaug)
    nlaug30 = np.concatenate([ha, ha, ma, ma, ha, la], 0)  # [30, T*L]
    recaug30 = np.concatenate([hb, mb, hb, mb, lb, hb], 0)  # [30, R]

    ligm = (np.arange(L) < int(lig_counts[b])).astype(f32)
    recm = (np.arange(R) < int(rec_counts[b])).astype(f32)

    fp8 = ml_dtypes.float8_e4m3fn
    lt = np.asarray(lig_feat[b], f32).transpose(1, 2, 0)  # [E,F,L]
    ligT = lt.reshape(E, KF, 128, L).transpose(2, 0, 1, 3)  # [128,E,KF,L]
    ligTb = np.ascontiguousarray(ligT[:, 0:4]).reshape(128, 4 * KF * L)
    ligTb = ligTb.astype(fp8)
    ligT4 = np.ascontiguousarray(ligT[:, 4]).reshape(128, KF * L)
    ligT4 = ligT4.astype(ml_dtypes.bfloat16)
    rt = np.asarray(rec_feat[b], f32).transpose(1, 2, 0)  # [E,F,R]
    recT = rt.reshape(E, KF, 128, R).transpose(2, 0, 1, 3)  # [128,E,KF,R]
    # rec mask pre-applied to the bf16 channels (so atn needs no device mask)
    recTb = np.ascontiguousarray(recT[:, 0:4] * recm).reshape(128, 4 * KF * R)
    recTb = recTb.astype(fp8)
    recT4 = np.ascontiguousarray(recT[:, 4] * recm).reshape(128, KF * R)
    recT4 = recT4.astype(ml_dtypes.bfloat16)

    # lig mask folded into nl2d columns (zeroes padded-l terms of U2)
    nl2d = np.empty((128, 5, T), f32)
    nl2d[:, 0:3, :] = (-2.0 * new_lig).transpose(1, 2, 0)
    nl2d[:, 3, :] = nl2.T
    nl2d[:, 4, :] = 1.0
    nl2d *= ligm[:, None, None]
    nl2d = nl2d.reshape(128, 5 * T)

    # rec mask folded into ydev rows (zeroes padded-r terms of W)
    y = np.empty((R, 5), f32)
    y[:, 0:3] = rc
    y[:, 3] = 1.0
    y[:, 4] = rec2
    y *= recm[:, None]
    ydev = np.ascontiguousarray(y.reshape(8, 128, 5).transpose(1, 0, 2)).reshape(
        128, 40
    )

    # lig mask folded into the one-hot reduction columns
    oh = np.zeros((128, T, T), f32)
    oh[:, np.arange(T), np.arange(T)] = ligm[:, None]
    onehot = oh.reshape(128, T * T).astype(ml_dtypes.bfloat16)

    return {
        "ligTb": ligTb,
        "ligT4": ligT4,
        "recTb": recTb,
        "recT4": recT4,
        "nlaug": nlaug30,
        "recaug": recaug30,
        "nl2d": nl2d,
        "ydev": ydev,
        "onehot": onehot,
    }


def host_rot(pre_rot):
    return np.linalg.qr(np.asarray(pre_rot, np.float32))[0]


# --------------------------------------------------------------------------
# entry point
# --------------------------------------------------------------------------
def kernel(
    lig_feat, rec_feat, lig_coord, rec_coord, pre_rot, trans, lig_counts, rec_counts
):
    global _BUILT
    from concourse.bass_utils import run_bass_kernel_spmd

    if _BUILT is None:
        _BUILT = build_nc()
    nc = _BUILT

    rot = host_rot(pre_rot)
    in_maps = [
        prep_core_inputs(
            b,
            lig_feat,
            rec_feat,
            lig_coord,
            rec_coord,
            rot,
            trans,
            lig_counts,
            rec_counts,
        )
        for b in range(B)
    ]
    res = run_bass_kernel_spmd(nc, in_maps, core_ids=list(range(NCHIP))).results
    out = np.empty((B, T), np.float32)
    for b in range(B):
        out[b] = res[b]["u4"][:, 0] + res[b]["u2"][0, :]
    return out
